# revision 43
# baseline (speedup 1.0000x reference)
"""3-block decoder (causal MHA + full MHA + 4-layer FFN, post-LN) on 8 TRN2 cores.

Sharding (SPMD-uniform across all 8 cores; core = 2*batch + hh):
  - Both MHA layers: Megatron tensor-parallel over heads within core pairs
    {2b, 2b+1} (8 heads each); AllReduce after layer-1's out-projection,
    ReduceScatter (alternating 512-token blocks) after layer-2's.
  - FFN + LN2/LN3 + output: token-split across the pair (each core does its
    alternating 512-blocks only). Host interleaves the two out halves.

Layout: activations feature-major [feat(P), tok] ("FM"), bf16 everywhere on
the x side; weights bf16 (host-converted). V and the softmax weights are fp8
(e4m3) so the AV matmuls run in DoubleRow perf mode (2 key-tiles contracted
per matmul at 0.5 cycles/row); the fp8 V scale (x32 layer1 / x4 layer2) is
folded into wv host-side and cancelled via 1/scale folded into wo.
Scores are computed transposed S^T=[k,q] in bf16; softmax denominators via a
ones-column appended to V; normalization deferred to the O tiles.

Fusions: embedding gather+transpose feeds layer-1 projections directly
(e stays in SBUF); LN1 feeds layer-2 projections directly (n1 stays in SBUF,
pre-scaled by ln_g); the layer-2 residual (n1*g + lnb)/2 is added into each
core's partial out-projection so the ReduceScatter output IS x2; the tail
(LN2+FFN+LN3+output transpose) is one fused pass per 256-token block.
"""
import sys, os

for p in ("/opt/trn_rl_repo", "/root/.axon_site/_ro/trn_rl_repo"):
    if os.path.isdir(p) and p not in sys.path:
        sys.path.insert(0, p)

import numpy as np
import ml_dtypes
import concourse.bass as bass
import concourse.bacc as bacc
import concourse.tile as tile
import concourse.mybir as mybir
from concourse import bass_utils
from concourse.masks import make_identity

F32 = mybir.dt.float32
F32R = mybir.dt.float32r
BF16 = mybir.dt.bfloat16
FP8 = mybir.dt.float8e4
I32 = mybir.dt.int32
DR = mybir.MatmulPerfMode.DoubleRow
Exp = mybir.ActivationFunctionType.Exp
Relu = mybir.ActivationFunctionType.Relu
Square = mybir.ActivationFunctionType.Square
Sqrt = mybir.ActivationFunctionType.Sqrt
ADD = mybir.AluOpType.add
SUB = mybir.AluOpType.subtract
MULT = mybir.AluOpType.mult

B, E, H, HD, V = 4, 1024, 16, 64, 32000
HL = 8          # local heads per core (H/2)
P = 128
NC = E // P     # 8 feature chunks
NCH = 512 // P  # 4 chunks of the local-head dim
EPS = 1e-5
VS1, VS2 = 32.0, 4.0   # fp8 V scales (folded into wv/wo host-side)


def build_program(T, n_cores, fake_cc=False, dbg=False):
    NB = T // 512
    NTT = T // P
    NBH = NB // 2
    groups = [[2 * i, 2 * i + 1] for i in range(n_cores // 2)]

    nc = bacc.Bacc("TRN2", target_bir_lowering=False, debug=False,
                   enable_asserts=False, num_devices=n_cores)

    dt_ = nc.dram_tensor
    ids = dt_("ids", [T, 1], I32, kind="ExternalInput").ap()
    tok_emb = dt_("tok_emb", [V, E], BF16, kind="ExternalInput").ap()
    pos_emb = dt_("pos_emb", [T, E], BF16, kind="ExternalInput").ap()
    wq1 = dt_("wq1", [E, 512], BF16, kind="ExternalInput").ap()
    wk1 = dt_("wk1", [E, 512], BF16, kind="ExternalInput").ap()
    wv1 = dt_("wv1", [E, 512], BF16, kind="ExternalInput").ap()
    wo1 = dt_("wo1", [512, E], BF16, kind="ExternalInput").ap()
    wq2 = dt_("wq2", [E, 512], BF16, kind="ExternalInput").ap()
    wk2 = dt_("wk2", [E, 512], BF16, kind="ExternalInput").ap()
    wv2 = dt_("wv2", [E, 512], BF16, kind="ExternalInput").ap()
    wo2 = dt_("wo2", [512, E], BF16, kind="ExternalInput").ap()
    fw1 = dt_("fw1", [E, 512], BF16, kind="ExternalInput").ap()
    fw2 = dt_("fw2", [512, E], BF16, kind="ExternalInput").ap()
    fw3 = dt_("fw3", [E, 512], BF16, kind="ExternalInput").ap()
    fw4 = dt_("fw4", [512, E], BF16, kind="ExternalInput").ap()
    bo1h = dt_("bo1h", [E], F32, kind="ExternalInput").ap()    # m_bo/2
    bo2h = dt_("bo2h", [E], F32, kind="ExternalInput").ap()    # (h_bo+ln_b)/2
    kb2 = dt_("kb2", [512], F32, kind="ExternalInput").ap()    # ln_b @ h_Wk slice
    qb2 = dt_("qb2", [512], F32, kind="ExternalInput").ap()
    vb2 = dt_("vb2", [512], F32, kind="ExternalInput").ap()    # 4*(ln_b @ h_Wv) slice
    fb1c = dt_("fb1c", [512], F32, kind="ExternalInput").ap()  # ln_b@f_W1+f_b1
    fb2c = dt_("fb2c", [E], F32, kind="ExternalInput").ap()
    fb3c = dt_("fb3c", [512], F32, kind="ExternalInput").ap()
    fb4c = dt_("fb4c", [E], F32, kind="ExternalInput").ap()    # f_b4 + ln_b
    gv = dt_("gv", [E], F32, kind="ExternalInput").ap()      # ln_g
    lnbv = dt_("lnbv", [E], F32, kind="ExternalInput").ap()  # ln_b
    masks = dt_("masks", [4, P, 512], BF16, kind="ExternalInput").ap()
    out = dt_("out", [T // 2, E], F32, kind="ExternalOutput").ap()
    if dbg:
        dbg_e = dt_("dbg_e", [P, NC, T], BF16, kind="ExternalOutput").ap()
        dbg_n1 = dt_("dbg_n1", [P, NC, T], BF16, kind="ExternalOutput").ap()
        dbg_k = [dt_(f"dbg_k{i}", [P, NCH, T], BF16, kind="ExternalOutput").ap()
                 for i in (1, 2)]
        dbg_q = [dt_(f"dbg_q{i}", [P, NCH, T], BF16, kind="ExternalOutput").ap()
                 for i in (1, 2)]
        dbg_v = [dt_(f"dbg_v{i}", [P, NTT // 2, HL, 2, 96], FP8,
                     kind="ExternalOutput").ap() for i in (1, 2)]
        dbg_af1 = dt_("dbg_af1", [P, NC, T], BF16, kind="ExternalOutput").ap()
        dbg_x2 = dt_("dbg_x2", [P, NC, T // 2], BF16, kind="ExternalOutput").ap()

    def cp(w):  # [K, N] -> [p, K/128, N] for chunked lhsT loads
        return w.rearrange("(c p) n -> p c n", p=P)

    def vchunk(v):  # [F] -> [p, F/128] (chunk-major per-partition consts)
        return v.rearrange("(c p) -> p c", p=P)

    with tile.TileContext(nc) as tc:
      with nc.allow_low_precision(reason="bf16/fp8 compute by design"), \
           tc.tile_pool(name="const", bufs=1) as const, \
           tc.tile_pool(name="dram", bufs=1, space="DRAM") as dram:

        # ---- constants resident in SBUF ----
        ident_f = const.tile([P, P], F32)
        make_identity(nc, ident_f[:])
        ident = const.tile([P, P], BF16)
        nc.vector.tensor_copy(ident[:], ident_f[:])
        ones_cb = const.tile([P, 1], BF16)
        nc.vector.memset(ones_cb[:], 1.0)
        ones_rf = const.tile([1, P], F32)
        nc.vector.memset(ones_rf[:], 1.0)
        ones_rr = const.tile([1, P], F32R)
        nc.vector.tensor_copy(ones_rr[:], ones_rf[:])
        ones_f8 = const.tile([P, 1], FP8)
        nc.vector.tensor_copy(ones_f8[:], ones_cb[:])
        eps_t = const.tile([1, 1], F32)
        nc.vector.memset(eps_t[:], EPS)
        g_sb = const.tile([P, NC], F32)
        nc.sync.dma_start(g_sb[:], vchunk(gv))
        lnb_sb = const.tile([P, NC], F32)
        nc.sync.dma_start(lnb_sb[:], vchunk(lnbv))
        bo1_sb = const.tile([P, NC], F32)
        nc.sync.dma_start(bo1_sb[:], vchunk(bo1h))
        bo2_sb = const.tile([P, NC], F32)
        nc.sync.dma_start(bo2_sb[:], vchunk(bo2h))
        kb2_sb = const.tile([P, NCH], F32)
        nc.sync.dma_start(kb2_sb[:], vchunk(kb2))
        qb2_sb = const.tile([P, NCH], F32)
        nc.sync.dma_start(qb2_sb[:], vchunk(qb2))
        fb1_sb = const.tile([P, NCH], F32)
        nc.sync.dma_start(fb1_sb[:], vchunk(fb1c))
        fb2_sb = const.tile([P, NC], F32)
        nc.sync.dma_start(fb2_sb[:], vchunk(fb2c))
        fb3_sb = const.tile([P, NCH], F32)
        nc.sync.dma_start(fb3_sb[:], vchunk(fb3c))
        fb4_sb = const.tile([P, NC], F32)
        nc.sync.dma_start(fb4_sb[:], vchunk(fb4c))
        # vb2 broadcast across partitions (per-feature bias of token-major V2)
        vb2_b = const.tile([P, HL * HD], F32)
        nc.sync.dma_start(vb2_b[:], bass.AP(tensor=vb2.tensor, offset=vb2.offset,
                                            ap=[[0, P], [1, HL * HD]]))
        mask_sb = const.tile([P, 4, 512], BF16)
        nc.sync.dma_start(mask_sb[:], masks.rearrange("m p q -> p m q"))

        # ---- internal DRAM (collective buffers) ----
        attn_p1 = [dram.tile([P, NC, 512], BF16, name=f"ap1_{q}")
                   for q in range(NB)]
        attn_f1 = [dram.tile([P, NC, 512], BF16, name=f"af1_{q}")
                   for q in range(NB)]
        attn_p2 = [dram.tile([2, P, NC, 512], BF16, name=f"ap2_{k}")
                   for k in range(NBH)]
        attn_f2 = [dram.tile([P, NC, 512], BF16, name=f"af2_{k}")
                   for k in range(NBH)]

        e_dram = dram.tile([P, NC, T], BF16, name="e_fm")

        # ---- persistent SBUF: n1 residual + per-layer K/Q/V (disjoint so
        # layer-2 projections can overlap layer-1 attention) ----
        with tc.tile_pool(name="persist", bufs=1) as persist:
            n1_sb = persist.tile([P, NC, T], BF16)   # LN1 out, pre-scaled by g
            K_sb = persist.tile([P, NCH, T], BF16)
            Q_sb = persist.tile([P, NCH, T], BF16)
            V8_sb = persist.tile([P, NTT // 2, HL, 2, 96], FP8)
            K2_sb = persist.tile([P, NCH, T], BF16)
            Q2_sb = persist.tile([P, NCH, T], BF16)
            V82_sb = persist.tile([P, NTT // 2, HL, 2, 96], FP8)
            for vt in (V8_sb, V82_sb):
                nc.vector.tensor_copy(
                    vt[:, :, :, :, HD:96],
                    ones_f8[:, None, :, None, None].to_broadcast(
                        [P, NTT // 2, HL, 2, 96 - HD]))

            # ======== helpers ==========================================
            def ln_g_pass(x, W, out_ap, lnp, lnps, with_lnb):
                """out = (x - mean)/std * g (+ lnb). x: [P, NC, W] bf16 SBUF.
                Stats via PE; rstd/mean broadcasts staged to SBUF bf16 (PSUM
                is DVE/Act-only on HW); per-chunk normalize split DVE/Pool
                with g/lnb applied as per-partition scalars."""
                s_ps = lnps.tile([1, W], F32, tag="s")
                q_ps = lnps.tile([1, W], F32, tag="q")
                for c in range(NC):
                    xsq = lnp.tile([P, W], BF16, tag="xsq", bufs=2)
                    nc.scalar.activation(xsq[:], x[:, c, :], Square)
                    nc.tensor.matmul(s_ps[:], lhsT=ones_cb[:], rhs=x[:, c, :],
                                     start=(c == 0), stop=(c == NC - 1))
                    nc.tensor.matmul(q_ps[:], lhsT=ones_cb[:], rhs=xsq[:],
                                     start=(c == 0), stop=(c == NC - 1))
                m_t = lnp.tile([1, W], F32, tag="m", bufs=1)
                nc.vector.tensor_scalar(m_t[:], s_ps[:], 1.0 / E, None, MULT)
                var_t = lnp.tile([1, W], F32, tag="var", bufs=1)
                nc.gpsimd.tensor_tensor(var_t[:], m_t[:], m_t[:], MULT)
                sd_t = lnp.tile([1, W], F32, tag="sd", bufs=1)
                nc.vector.tensor_scalar(sd_t[:], q_ps[:], 1.0 / E, None, MULT)
                nc.gpsimd.tensor_tensor(var_t[:], sd_t[:], var_t[:], SUB)
                nc.scalar.activation(sd_t[:], var_t[:], Sqrt, bias=eps_t[:, 0:1])
                rstd_t = lnp.tile([1, W], F32R, tag="rstd", bufs=1)
                nc.vector.reciprocal(rstd_t[:], sd_t[:])
                mr_t = lnp.tile([1, W], F32R, tag="mr", bufs=1)
                nc.gpsimd.tensor_tensor(mr_t[:], m_t[:], rstd_t[:], MULT)
                rb_ps = lnps.tile([P, W], F32, tag="bc")
                nc.tensor.matmul(rb_ps[:], lhsT=ones_rr[0:1, :],
                                 rhs=rstd_t[:], start=True, stop=True)
                rb_sb = lnp.tile([P, W], BF16, tag="rb", bufs=2)
                nc.scalar.copy(rb_sb[:], rb_ps[:])
                mb_ps = lnps.tile([P, W], F32, tag="bc")
                nc.tensor.matmul(mb_ps[:], lhsT=ones_rr[0:1, :],
                                 rhs=mr_t[:], start=True, stop=True)
                mb_sb = lnp.tile([P, W], BF16, tag="mb", bufs=2)
                nc.scalar.copy(mb_sb[:], mb_ps[:])
                for c in range(NC):
                    eng = nc.vector if c % 2 else nc.gpsimd
                    t_t = lnp.tile([P, W], BF16, tag="t", bufs=4)
                    eng.tensor_tensor(t_t[:], x[:, c, :], rb_sb[:], MULT)
                    eng.tensor_tensor(t_t[:], t_t[:], mb_sb[:], SUB)
                    if with_lnb:
                        eng.tensor_scalar(out_ap[:, c, :], t_t[:],
                                          g_sb[:, c:c + 1], lnb_sb[:, c:c + 1],
                                          MULT, ADD)
                    else:
                        eng.tensor_scalar(out_ap[:, c, :], t_t[:],
                                          g_sb[:, c:c + 1], None, MULT)

            def proj_block(xsrc, bb, w3, K_sb, Q_sb, V8_sb, kbias, qbias,
                           use_vbias, pps, pst):
                """One 512-token block of K/Q (FM, SBUF) + V (fp8 TM, SBUF)."""
                wk_sb, wq_sb, wv_sb = w3
                tb = slice(bb * 512, (bb + 1) * 512)
                for m in range(NCH):
                    ps = pps.tile([P, 512], F32, tag="bank")
                    for c in range(NC):
                        nc.tensor.matmul(ps[:], lhsT=wk_sb[:, c, m * P:(m + 1) * P],
                                         rhs=xsrc[:, c, :], start=(c == 0),
                                         stop=(c == NC - 1))
                    if kbias is None:
                        nc.vector.tensor_copy(K_sb[:, m, tb], ps[:])
                    else:
                        nc.vector.tensor_scalar(K_sb[:, m, tb], ps[:],
                                                kbias[:, m:m + 1], None, ADD)
                    ps = pps.tile([P, 512], F32, tag="bank")
                    for c in range(NC):
                        nc.tensor.matmul(ps[:], lhsT=wq_sb[:, c, m * P:(m + 1) * P],
                                         rhs=xsrc[:, c, :], start=(c == 0),
                                         stop=(c == NC - 1))
                    if qbias is None:
                        nc.scalar.copy(Q_sb[:, m, tb], ps[:])
                    else:
                        nc.scalar.activation(
                            Q_sb[:, m, tb], ps[:],
                            mybir.ActivationFunctionType.Identity,
                            bias=qbias[:, m:m + 1])
                for st_i in range(4):
                    tt = bb * 4 + st_i
                    ps = pps.tile([P, 512], F32, tag="bank")
                    for c in range(NC):
                        nc.tensor.matmul(
                            ps[:], lhsT=xsrc[:, c, st_i * P:(st_i + 1) * P],
                            rhs=wv_sb[:, c, :], start=(c == 0), stop=(c == NC - 1))
                    dst = V8_sb[:, tt // 2, :, tt % 2, 0:HD]
                    src = ps[:].rearrange("p (h d) -> p h d", h=HL)
                    if use_vbias:
                        nc.vector.tensor_tensor(
                            dst, src, vb2_b[:].rearrange("p (h d) -> p h d", h=HL),
                            ADD)
                    else:
                        nc.vector.tensor_copy(dst, src)

            def attention(K_sb, Q_sb, V8_sb, wo_sb, causal, epilogue, post_qb):
                with tc.tile_pool(name="ao", bufs=1) as ao, \
                     tc.tile_pool(name="ast", bufs=3) as ast, \
                     tc.tile_pool(name="aat", bufs=1) as aat, \
                     tc.tile_pool(name="asc", bufs=2, space="PSUM") as asc, \
                     tc.tile_pool(name="aav", bufs=1, space="PSUM") as aav, \
                     tc.tile_pool(name="abc", bufs=2, space="PSUM") as abc:
                    for qb in range(NB):
                        n_kt = (4 * qb + 4) if causal else NTT
                        o_blk = ao.tile([P, NCH, 512], BF16, tag="oblk", bufs=2)
                        for hp in range(NCH):
                            av = [aav.tile([96, 512], F32, tag=f"av{j}",
                                           name=f"av{qb}_{hp}_{j}")
                                  for j in (0, 1)]
                            at8 = None
                            for kt in range(n_kt):
                                sps = asc.tile([P, 2, 512], F32, tag="sc")
                                for j in (0, 1):
                                    o = j * HD
                                    nc.tensor.matmul(
                                        sps[:, j, :],
                                        lhsT=K_sb[o:o + HD, hp, kt * P:(kt + 1) * P],
                                        rhs=Q_sb[o:o + HD, hp,
                                                 qb * 512:(qb + 1) * 512],
                                        start=True, stop=True)
                                if causal and kt >= 4 * qb:
                                    r = kt - 4 * qb
                                    nc.vector.tensor_tensor(
                                        sps[:], sps[:],
                                        mask_sb[:, r:r + 1, :].to_broadcast(
                                            [P, 2, 512]), ADD)
                                if kt % 2 == 0:
                                    at8 = aat.tile([P, 2, 2, 512], FP8,
                                                   tag="at8", bufs=2)
                                nc.scalar.activation(at8[:, kt % 2, :, :], sps[:],
                                                     Exp, scale=0.125)
                                if kt % 2 == 1:
                                    for j in (0, 1):
                                        nc.tensor.matmul(
                                            av[j][:],
                                            lhsT=V8_sb[:, kt // 2,
                                                       2 * hp + j, :, :],
                                            rhs=at8[:, :, j, :],
                                            start=(kt == 1),
                                            stop=(kt == n_kt - 1),
                                            perf_mode=DR)
                            for j in (0, 1):
                                rd = ast.tile([1, 512], F32R, tag="rd")
                                nc.vector.reciprocal(rd[:], av[j][64:65, :])
                                bc_ps = abc.tile([64, 512], F32, tag="bcp")
                                nc.tensor.matmul(bc_ps[:],
                                                 lhsT=ones_rr[0:1, 0:64],
                                                 rhs=rd[:], start=True, stop=True)
                                bcs = ast.tile([64, 512], BF16, tag="bc")
                                nc.vector.tensor_copy(bcs[:], bc_ps[:])
                                if j == 0:
                                    nc.vector.tensor_tensor(
                                        o_blk[0:64, hp, :], av[j][0:64, :],
                                        bcs[:], MULT)
                                else:
                                    ot = ast.tile([64, 512], BF16, tag="ot")
                                    nc.vector.tensor_tensor(
                                        ot[:], av[j][0:64, :], bcs[:], MULT)
                                    nc.sync.dma_start(o_blk[64:128, hp, :], ot[:])
                        for m in range(NC):
                            ps_t = asc.tile([P, 2, 512], F32, tag="sc",
                                            name=f"op{qb}_{m}")
                            ps = ps_t[:, 0, :]
                            for c in range(NCH):
                                nc.tensor.matmul(
                                    ps, lhsT=wo_sb[:, c, m * P:(m + 1) * P],
                                    rhs=o_blk[:, c, :], start=(c == 0),
                                    stop=(c == NCH - 1))
                            epilogue(qb, m, ps, ast)
                        post_qb(qb)

            # ================= embeddings fused into layer-1 proj ==========
            with tc.tile_pool(name="l1", bufs=1) as l1:
                K1, Q1, V81 = K_sb, Q_sb, V8_sb
                with tc.tile_pool(name="w1", bufs=1) as w1p, \
                     tc.tile_pool(name="emb", bufs=3) as emb_p, \
                     tc.tile_pool(name="pst1", bufs=3) as pst1, \
                     tc.tile_pool(name="ps1", bufs=4, space="PSUM") as ps1, \
                     tc.tile_pool(name="tps", bufs=4, space="PSUM") as tps:
                    wk_sb = w1p.tile([P, NC, 512], BF16)
                    nc.sync.dma_start(wk_sb[:], cp(wk1))
                    wq_sb = w1p.tile([P, NC, 512], BF16)
                    nc.sync.dma_start(wq_sb[:], cp(wq1))
                    wv_sb = w1p.tile([P, NC, 512], BF16)
                    nc.sync.dma_start(wv_sb[:], cp(wv1))
                    for bb in range(NB):
                        e_blk = emb_p.tile([P, NC, 512], BF16, tag="eblk",
                                           bufs=2)
                        for st_i in range(4):
                            tt = bb * 4 + st_i
                            ids_t = emb_p.tile([P, 1], I32, tag="ids")
                            nc.sync.dma_start(ids_t[:], ids[tt * P:(tt + 1) * P, :])
                            et = emb_p.tile([P, E], BF16, tag="emb")
                            nc.gpsimd.indirect_dma_start(
                                out=et[:], out_offset=None, in_=tok_emb[:],
                                in_offset=bass.IndirectOffsetOnAxis(
                                    ap=ids_t[:, :1], axis=0))
                            pt = emb_p.tile([P, E], BF16, tag="pos")
                            nc.sync.dma_start(pt[:], pos_emb[tt * P:(tt + 1) * P, :])
                            nc.vector.tensor_tensor(et[:], et[:], pt[:], ADD)
                            for c in range(NC):
                                tp = tps.tile([P, P], BF16, tag="tp")
                                nc.tensor.transpose(tp[:], et[:, c * P:(c + 1) * P],
                                                    ident[:])
                                nc.scalar.copy(
                                    e_blk[:, c, st_i * P:(st_i + 1) * P], tp[:])
                        nc.sync.dma_start(
                            e_dram[:, :, bb * 512:(bb + 1) * 512], e_blk[:])
                        if dbg:
                            nc.sync.dma_start(
                                dbg_e[:, :, bb * 512:(bb + 1) * 512], e_blk[:])
                        proj_block(e_blk[:], bb,
                                   (wk_sb, wq_sb, wv_sb), K1, Q1, V81,
                                   None, None, False, ps1, pst1)
                if dbg:
                    nc.sync.dma_start(dbg_k[0], K1[:])
                    nc.sync.dma_start(dbg_q[0], Q1[:])
                    nc.sync.dma_start(dbg_v[0], V81[:])

                # ---- layer-1 attention (causal) + AllReduce ----
                def epi1(qb, m, ps, ast):
                    st = ast.tile([P, 512], BF16, tag="st")
                    nc.vector.tensor_scalar(st[:], ps, bo1_sb[:, m:m + 1],
                                            None, ADD)
                    nc.sync.dma_start(attn_p1[qb][:, m, :], st[:])

                def post1(qb):
                    _cc_ar(nc, tc, attn_p1[qb], attn_f1[qb], groups, fake_cc)

                with tc.tile_pool(name="wo1p", bufs=1) as wo1p:
                    wo_sb = wo1p.tile([P, NCH, E], BF16)
                    nc.sync.dma_start(wo_sb[:], cp(wo1))
                    attention(K1, Q1, V81, wo_sb, True, epi1, post1)

                # ================= LN1 fused into layer-2 proj =============
                with tc.tile_pool(name="l2", bufs=1) as l2:
                    K2, Q2, V82 = K2_sb, Q2_sb, V82_sb
                    with tc.tile_pool(name="w2", bufs=1) as w2p, \
                         tc.tile_pool(name="ln1", bufs=2) as lnp, \
                         tc.tile_pool(name="pst2", bufs=3) as pst2, \
                         tc.tile_pool(name="ps2", bufs=3, space="PSUM") as ps2, \
                         tc.tile_pool(name="lnps1", bufs=1, space="PSUM") as lnps:
                        wk2_sb = w2p.tile([P, NC, 512], BF16)
                        nc.sync.dma_start(wk2_sb[:], cp(wk2))
                        wq2_sb = w2p.tile([P, NC, 512], BF16)
                        nc.sync.dma_start(wq2_sb[:], cp(wq2))
                        wv2_sb = w2p.tile([P, NC, 512], BF16)
                        nc.sync.dma_start(wv2_sb[:], cp(wv2))
                        for bb in range(NB):
                            tb = slice(bb * 512, (bb + 1) * 512)
                            x1 = lnp.tile([P, NC, 512], BF16, tag="x1")
                            nc.sync.dma_start(x1[:], attn_f1[bb][:])
                            eb = lnp.tile([P, NC, 512], BF16, tag="eb")
                            nc.sync.dma_start(eb[:], e_dram[:, :, tb])
                            nc.vector.tensor_tensor(x1[:], eb[:], x1[:], ADD)
                            if dbg:
                                nc.sync.dma_start(dbg_af1[:, :, tb], x1[:])
                            ln_g_pass(x1[:], 512, n1_sb[:, :, tb], lnp, lnps,
                                      with_lnb=False)
                            proj_block(n1_sb[:, :, tb], bb,
                                       (wk2_sb, wq2_sb, wv2_sb), K2, Q2, V82,
                                       kb2_sb, qb2_sb, True, ps2, pst2)
                        if dbg:
                            nc.sync.dma_start(dbg_n1, n1_sb[:])
                            nc.sync.dma_start(dbg_k[1], K2[:])
                            nc.sync.dma_start(dbg_q[1], Q2[:])
                            nc.sync.dma_start(dbg_v[1], V82[:])

                    # ---- layer-2 attention (full) + ReduceScatter ----
                    def epi2(qb, m, ps, ast):
                        tb = slice(qb * 512, (qb + 1) * 512)
                        tmp = ast.tile([P, 512], BF16, tag="tmp")
                        nc.gpsimd.tensor_scalar(tmp[:], n1_sb[:, m, tb], 0.5,
                                                bo2_sb[:, m:m + 1], MULT, ADD)
                        st = ast.tile([P, 512], BF16, tag="st")
                        nc.vector.tensor_tensor(st[:], ps, tmp[:], ADD)
                        nc.sync.dma_start(attn_p2[qb // 2][qb % 2, :, m, :], st[:])

                    def post2(qb):
                        if qb % 2 == 1:
                            _cc_rs(nc, tc, attn_p2[qb // 2], attn_f2[qb // 2],
                                   groups, fake_cc)

                    with tc.tile_pool(name="wo2p", bufs=1) as wo2p:
                        wo2_sb = wo2p.tile([P, NCH, E], BF16)
                        nc.sync.dma_start(wo2_sb[:], cp(wo2))
                        attention(K2, Q2, V82, wo2_sb, False, epi2, post2)

            # ========== tail: LN2 + FFN + LN3 + out, per 256-token block ====
            with tc.tile_pool(name="fw", bufs=1) as fwp, \
                 tc.tile_pool(name="ff", bufs=1) as ffp, \
                 tc.tile_pool(name="ffps", bufs=3, space="PSUM") as ffps, \
                 tc.tile_pool(name="lnps2", bufs=1, space="PSUM") as lnps2, \
                 tc.tile_pool(name="otps", bufs=2, space="PSUM") as otps:
                fw1_sb = fwp.tile([P, NC, 512], BF16)
                nc.sync.dma_start(fw1_sb[:], cp(fw1))
                fw2_sb = fwp.tile([P, NCH, E], BF16)
                nc.sync.dma_start(fw2_sb[:], cp(fw2))
                fw3_sb = fwp.tile([P, NC, 512], BF16)
                nc.sync.dma_start(fw3_sb[:], cp(fw3))
                fw4_sb = fwp.tile([P, NCH, E], BF16)
                nc.sync.dma_start(fw4_sb[:], cp(fw4))
                W = 256
                for w in range((T // 2) // W):
                    hs = slice((w % 2) * W, (w % 2 + 1) * W)
                    x2 = ffp.tile([P, NC, W], BF16, tag="x2", bufs=2)
                    nc.sync.dma_start(x2[:], attn_f2[w // 2][:, :, hs])
                    if dbg:
                        nc.sync.dma_start(dbg_x2[:, :, w * W:(w + 1) * W], x2[:])
                    n2g = ffp.tile([P, NC, W], BF16, tag="n2g", bufs=2)
                    ln_g_pass(x2[:], W, n2g[:], ffp, lnps2, with_lnb=False)
                    h1 = ffp.tile([P, NCH, W], BF16, tag="hsm", bufs=2)
                    for m in range(NCH):
                        ps = ffps.tile([P, W], F32, tag="bank")
                        for c in range(NC):
                            nc.tensor.matmul(ps[:], lhsT=fw1_sb[:, c, m * P:(m + 1) * P],
                                             rhs=n2g[:, c, :], start=(c == 0),
                                             stop=(c == NC - 1))
                        nc.scalar.activation(h1[:, m, :], ps[:], Relu,
                                             bias=fb1_sb[:, m:m + 1])
                    h2 = ffp.tile([P, NC, W], BF16, tag="h2", bufs=2)
                    for m in range(NC):
                        ps = ffps.tile([P, W], F32, tag="bank")
                        for c in range(NCH):
                            nc.tensor.matmul(ps[:], lhsT=fw2_sb[:, c, m * P:(m + 1) * P],
                                             rhs=h1[:, c, :], start=(c == 0),
                                             stop=(c == NCH - 1))
                        nc.scalar.activation(h2[:, m, :], ps[:], Relu,
                                             bias=fb2_sb[:, m:m + 1])
                    h3 = ffp.tile([P, NCH, W], BF16, tag="hsm", bufs=2)
                    for m in range(NCH):
                        ps = ffps.tile([P, W], F32, tag="bank")
                        for c in range(NC):
                            nc.tensor.matmul(ps[:], lhsT=fw3_sb[:, c, m * P:(m + 1) * P],
                                             rhs=h2[:, c, :], start=(c == 0),
                                             stop=(c == NC - 1))
                        nc.scalar.activation(h3[:, m, :], ps[:], Relu,
                                             bias=fb3_sb[:, m:m + 1])
                    x3 = ffp.tile([P, NC, W], BF16, tag="x2", bufs=2)
                    for m in range(NC):
                        ps = ffps.tile([P, W], F32, tag="bank")
                        for c in range(NCH):
                            nc.tensor.matmul(ps[:], lhsT=fw4_sb[:, c, m * P:(m + 1) * P],
                                             rhs=h3[:, c, :], start=(c == 0),
                                             stop=(c == NCH - 1))
                        nc.vector.tensor_scalar(x3[:, m, :], ps[:],
                                                fb4_sb[:, m:m + 1], None, ADD)
                        nc.gpsimd.tensor_tensor(x3[:, m, :], x3[:, m, :],
                                                n2g[:, m, :], ADD)
                    r3 = ffp.tile([P, NC, W], BF16, tag="n2g", bufs=2)
                    ln_g_pass(x3[:], W, r3[:], ffp, lnps2, with_lnb=True)
                    for c in range(NC):
                        for s in range(W // P):
                            tp = otps.tile([P, P], BF16, tag="otp")
                            nc.tensor.transpose(tp[:], r3[:, c, s * P:(s + 1) * P],
                                                ident[:])
                            ost = ffp.tile([P, P], F32, tag="ost", bufs=4)
                            nc.scalar.copy(ost[:], tp[:])
                            nc.sync.dma_start(
                                out[w * W + s * P: w * W + (s + 1) * P,
                                    c * P:(c + 1) * P], ost[:])

    nc.compile()
    return nc


def _cc_ar(nc, tc, src, dst, groups, fake_cc):
    if fake_cc:
        with tc.tile_pool(name="fcc", bufs=2) as fcc:
            st = fcc.tile([P, NC, 512], BF16, tag="st")
            nc.sync.dma_start(st[:], src[:])
            nc.sync.dma_start(dst[:], st[:])
        return
    nc.gpsimd.collective_compute(
        "AllReduce", mybir.AluOpType.add, replica_groups=groups,
        ins=[src[:].opt()], outs=[dst[:].opt()])


def _cc_rs(nc, tc, src, dst, groups, fake_cc):
    if fake_cc:
        with tc.tile_pool(name="fccr", bufs=2) as fcc:
            st = fcc.tile([P, NC, 512], BF16, tag="st")
            nc.sync.dma_start(st[:], src[0, :, :, :])
            nc.sync.dma_start(dst[:], st[:])
        return
    nc.gpsimd.collective_compute(
        "ReduceScatter", mybir.AluOpType.add, replica_groups=groups,
        ins=[src[:].opt()], outs=[dst[:].opt()])


_cache = {}


def _get_program(T, n_cores):
    key = (T, n_cores)
    if key not in _cache:
        _cache[key] = build_program(T, n_cores)
    return _cache[key]


def make_masks():
    m = np.zeros((4, P, 512), np.float32)
    for r in range(4):
        for k in range(P):
            m[r, k, :] = np.where(np.arange(512) >= (128 * r + k), 0.0, -1e9)
    return m


def build_in_maps(inputs, T, n_cores):
    BF = ml_dtypes.bfloat16
    f = lambda k: np.asarray(inputs[k], dtype=np.float32)
    bf = lambda a: np.ascontiguousarray(np.asarray(a, dtype=BF))
    x = np.asarray(inputs["x"]).astype(np.int32)
    ln_g, ln_b = f("ln_g"), f("ln_b")
    tok_emb = bf(f("tok_emb"))
    pos_emb = bf(f("pos_emb")[:T])
    masks = bf(make_masks())
    in_maps = []
    for core in range(n_cores):
        b, hh = core // 2, core % 2
        hs = slice(hh * 512, (hh + 1) * 512)
        im = dict(
            ids=x[b, :T].reshape(T, 1),
            tok_emb=tok_emb,
            pos_emb=pos_emb,
            wq1=bf(f("m_Wq")[:, hs]),
            wk1=bf(f("m_Wk")[:, hs]),
            wv1=bf(VS1 * f("m_Wv")[:, hs]),
            wo1=bf(f("m_Wo")[hs, :] / VS1),
            wq2=bf(f("h_Wq")[:, hs]),
            wk2=bf(f("h_Wk")[:, hs]),
            wv2=bf(VS2 * f("h_Wv")[:, hs]),
            wo2=bf(f("h_Wo")[hs, :] / VS2),
            fw1=bf(f("f_W1")), fw2=bf(f("f_W2")),
            fw3=bf(f("f_W3")), fw4=bf(f("f_W4")),
            bo1h=f("m_bo") / 2.0,
            bo2h=(f("h_bo") + ln_b) / 2.0,
            kb2=np.ascontiguousarray(ln_b @ f("h_Wk"))[hs],
            qb2=np.ascontiguousarray(ln_b @ f("h_Wq"))[hs],
            vb2=VS2 * np.ascontiguousarray(ln_b @ f("h_Wv"))[hs],
            fb1c=(ln_b @ f("f_W1") + f("f_b1")),
            fb2c=f("f_b2"), fb3c=f("f_b3"),
            fb4c=(f("f_b4") + ln_b),
            gv=ln_g, lnbv=ln_b,
            masks=masks,
        )
        in_maps.append({k: np.ascontiguousarray(v) for k, v in im.items()})
    return in_maps


def run(inputs, T=2048, n_cores=8):
    nc = _get_program(T, n_cores)
    in_maps = build_in_maps(inputs, T, n_cores)
    res = bass_utils.run_bass_kernel_spmd(nc, in_maps,
                                          core_ids=list(range(n_cores)))
    nb = n_cores // 2
    out = np.empty((nb, T, E), np.float32)
    for b in range(nb):
        ev = res.results[2 * b]["out"]
        od = res.results[2 * b + 1]["out"]
        for k in range(T // 1024):
            out[b, 2 * k * 512:(2 * k + 1) * 512] = ev[k * 512:(k + 1) * 512]
            out[b, (2 * k + 1) * 512:(2 * k + 2) * 512] = od[k * 512:(k + 1) * 512]
    return out, res


def kernel(**inputs):
    out, _ = run(inputs, T=2048, n_cores=8)
    return out
